# revision 1
# baseline (speedup 1.0000x reference)
"""Trainium2 Bass kernel for nn_Attention1 (dense transformer attention block).

Reference computation (per batch b):
  qkv = x @ w_in.T + b_in ; split q,k,v
  RoPE on first 64 channels of q and k (interleaved-pair rotate_half)
  16-head attention with key-padding mask, softmax, out-proj, mask-zeroed output.

Sharding (8 cores): data-parallel over batch (4) x tensor-parallel over
head-groups (2 groups of 8 heads). Each core computes its batch's QKV for its
head group, attention for 8 heads, and a partial out-projection over its 512
attention channels. The host sums the two head-group partials per batch
(the "all-reduce"), adds b_out, and zeroes masked positions.

All layouts are chosen so no on-device transposes are needed:
  xT [dim, n], qkT [ch, n], v [n, ch], E=exp(scores) [j, i], attn-out [ch, n].
The softmax denominator comes free from a ones-column appended to v (M=65
matmul). The key-padding mask is folded into v (and the ones column), so
exp needs no per-key bias; that lets score blocks of different key chunks
share one big exp op (3 blocks per [128,1536] PSUM tile), minimizing work
on the ScalarE/ACT engine — the true hardware bottleneck (~1 elem/lane/cyc
for the 33.5M-element exp per core). Matmuls run in bf16 (fp32 PSUM
accumulation); score K=64 matmuls are row-packed two-heads-per-pass via
PE tile_position inference.
"""

import math
from contextlib import ExitStack

import numpy as np
import ml_dtypes

import concourse.bass as bass
import concourse.tile as tile
from concourse import bacc, mybir
from concourse.bass_utils import run_bass_kernel_spmd

# Problem constants (hardcoded per harness contract)
B, N, DIM = 4, 2048, 1024
HEADS, DH = 16, 64
INNER = HEADS * DH          # 1024
NCORES = 8
HPG = 8                     # heads per group (2 groups)
CH = HPG * DH               # 512 channels per head group
P = 128
KD = DIM // P               # 8 contraction chunks
NJ = N // P                 # 16 key chunks
IB = 512                    # i-block (query block) size
NI = N // IB                # 4 query blocks
F32 = mybir.dt.float32
AFT = mybir.ActivationFunctionType

MASK_NEG = -1e9
DEPTH = 1      # attention j-loop software-pipeline depth


def _build_program(mmdt=mybir.dt.bfloat16):
    nc = bacc.Bacc("TRN2", debug=False)

    xT_d = nc.dram_tensor("xT", [DIM, N], mmdt, kind="ExternalInput").ap()
    wqkT_d = nc.dram_tensor("wqkT", [DIM, 2 * CH], mmdt, kind="ExternalInput").ap()
    wvT_d = nc.dram_tensor("wvT", [DIM, CH], mmdt, kind="ExternalInput").ap()
    woT_d = nc.dram_tensor("woT", [CH, DIM], mmdt, kind="ExternalInput").ap()
    fsin_d = nc.dram_tensor("fsin", [DH, N], F32, kind="ExternalInput").ap()
    fcos_d = nc.dram_tensor("fcos", [DH, N], F32, kind="ExternalInput").ap()
    rt_d = nc.dram_tensor("rt", [DH, DH], mmdt, kind="ExternalInput").ap()
    mb_d = nc.dram_tensor("mb", [P, NJ], F32, kind="ExternalInput").ap()
    bqk_d = nc.dram_tensor("bqk", [P, KD], F32, kind="ExternalInput").ap()
    bv_d = nc.dram_tensor("bv", [1, CH], F32, kind="ExternalInput").ap()
    out_d = nc.dram_tensor("out", [N, DIM], F32, kind="ExternalOutput").ap()

    with ExitStack() as ctx:
        tc = ctx.enter_context(tile.TileContext(nc))

        const = ctx.enter_context(tc.tile_pool(name="const", bufs=1))
        persist = ctx.enter_context(tc.tile_pool(name="persist", bufs=1))

        # ---- constant / persistent loads (xT/wv first: first compute
        #      needs them; fs/fc/wo are needed much later) ----
        rt_sb = const.tile([DH, DH], mmdt, tag="rt", name="rt")
        nc.sync.dma_start(out=rt_sb, in_=rt_d)
        mb_sb = const.tile([P, NJ], F32, tag="mb", name="mb")
        nc.sync.dma_start(out=mb_sb, in_=mb_d)
        bqk_sb = const.tile([P, KD], F32, tag="bqk", name="bqk")
        nc.sync.dma_start(out=bqk_sb, in_=bqk_d)
        # row of ones at partition 64 (lhsT for the denominator broadcast)
        ones_sb = const.tile([DH + 1, DH], F32, tag="ones", name="ones")
        nc.vector.memset(ones_sb[DH:DH + 1, :], 1.0)
        # broadcast v-bias to all 128 partitions via DMA with partition-step 0
        bv_sb = const.tile([P, CH], F32, tag="bv", name="bv")
        bv_bcast = bass.AP(tensor=bv_d.tensor, offset=bv_d.offset,
                           ap=[[0, P], [1, CH]])
        nc.gpsimd.dma_start(out=bv_sb, in_=bv_bcast)

        xT_sb = []
        wqk_sb = []
        wv_sb = []
        for k in range(KD):
            t = persist.tile([P, N], mmdt, tag=f"xT{k}", name=f"xT{k}")
            nc.sync.dma_start(out=t, in_=xT_d[k * P:(k + 1) * P, :])
            xT_sb.append(t)
            t = persist.tile([P, 2 * CH], mmdt, tag=f"wqk{k}", name=f"wqk{k}")
            nc.sync.dma_start(out=t, in_=wqkT_d[k * P:(k + 1) * P, :])
            wqk_sb.append(t)
        for k in range(KD):
            t = persist.tile([P, CH], mmdt, tag=f"wv{k}", name=f"wv{k}")
            nc.sync.dma_start(out=t, in_=wvT_d[k * P:(k + 1) * P, :])
            wv_sb.append(t)
        fs_sb = const.tile([DH, N], F32, tag="fs", name="fs")
        nc.sync.dma_start(out=fs_sb, in_=fsin_d)
        fc_sb = const.tile([DH, N], F32, tag="fc", name="fc")
        nc.sync.dma_start(out=fc_sb, in_=fcos_d)
        sin_sb = const.tile([DH, N], mmdt, tag="sin", name="sin")
        nc.scalar.activation(sin_sb, fs_sb, AFT.Sin)
        cos_sb = const.tile([DH, N], mmdt, tag="cos", name="cos")
        nc.scalar.activation(cos_sb, fc_sb, AFT.Sin)
        wo_sb = []
        for c in range(CH // P):
            t = persist.tile([P, DIM], mmdt, tag=f"wo{c}", name=f"wo{c}")
            nc.sync.dma_start(out=t, in_=woT_d[c * P:(c + 1) * P, :])
            wo_sb.append(t)

        # ---- phase 1: QKV projections ----
        v_sb = []       # 16 tiles [128 j, 8 heads, 65] (col 64 = ones for denom)
        qk_sb = []      # 8 tiles [128 ch, N]; 0-3 = q head-pairs, 4-7 = k
        for m in range(KD):
            qk_sb.append(persist.tile([P, N], mmdt, tag=f"qk{m}", name=f"qk{m}"))

        qk_emitter = {}
        with tc.tile_pool(name="ps1", bufs=2, space="PSUM") as ps1, \
             tc.tile_pool(name="rope", bufs=2) as rp_pool:

            def emit_qk_block(m, ib, pool=None):
                # RoPE (global head 0; identity when fsin/fcos encode freq 0)
                # fused right after chunks 0 / 4 so pair 0 unblocks first
                if True:
                    blk = slice(ib * IB, (ib + 1) * IB)
                    if pool is None:
                        qp = ps1.tile([P, IB], F32, tag="mm1", name="mm1")
                    else:
                        # phase-2 emission: borrow a score-tile slot
                        qp3 = pool.tile([P, 3 * IB], F32, tag="st3",
                                        name="qp3", bufs=2)
                        qp = qp3[:, 0:IB]
                    for k in range(KD):
                        nc.tensor.matmul(qp,
                                         lhsT=wqk_sb[k][:, m * P:(m + 1) * P],
                                         rhs=xT_sb[k][:, blk],
                                         start=(k == 0), stop=(k == KD - 1))
                    # copy with per-channel bias (b_in) fused on DVE
                    nc.vector.tensor_scalar_add(qk_sb[m][:, blk],
                                                qp, bqk_sb[:, m:m + 1])
                    if m in (0, 4):
                        rp = ps1.tile([DH, IB], F32, tag="ropeps",
                                      name="ropeps")
                        nc.tensor.matmul(rp, lhsT=rt_sb,
                                         rhs=qk_sb[m][0:DH, blk],
                                         start=True, stop=True)
                        t1 = rp_pool.tile([DH, IB], mmdt, tag="t1", name="t1")
                        nc.vector.tensor_mul(t1, rp, sin_sb[:, blk])
                        t2 = rp_pool.tile([DH, IB], mmdt, tag="t2", name="t2")
                        nc.vector.tensor_mul(t2, qk_sb[m][0:DH, blk],
                                             cos_sb[:, blk])
                        nc.vector.tensor_add(qk_sb[m][0:DH, blk], t1, t2)

            def emit_qk(m, pool=None):
                for ib in range(NI):
                    emit_qk_block(m, ib, pool)

            qk_emitter["f"] = emit_qk_block
            emit_qk(0)
            emit_qk(4)
            for j in range(NJ):
                vp = ps1.tile([P, CH], F32, tag="mm1", name="mm1")
                for k in range(KD):
                    nc.tensor.matmul(vp, lhsT=xT_sb[k][:, j * P:(j + 1) * P],
                                     rhs=wv_sb[k], start=(k == 0),
                                     stop=(k == KD - 1))
                vt = persist.tile([P, HPG, DH + 1], mmdt, tag=f"v{j}", name=f"v{j}")
                nc.vector.tensor_add(
                    vt[:, :, 0:DH],
                    vp.rearrange("p (h d) -> p h d", h=HPG),
                    bv_sb.rearrange("p (h d) -> p h d", h=HPG))
                nc.vector.memset(vt[:, :, DH:DH + 1], 1.0)
                # fold the key-padding mask into v and the denominator ones
                # column: masked keys contribute E*0, exactly like exp(-1e9)
                nc.vector.tensor_scalar_mul(
                    vt.rearrange("p h d -> p (h d)"),
                    vt.rearrange("p h d -> p (h d)"),
                    mb_sb[:, j:j + 1])
                v_sb.append(vt)

        # ---- phase 2+3: attention (iblk outer so the out-projection of
        #      each query block overlaps the next block's ACT-bound work) ----
        attnoutT = []
        for p in range(4):
            attnoutT.append(persist.tile([P, N], mmdt, tag=f"ao{p}", name=f"ao{p}"))

        with tc.tile_pool(name="ps_st", bufs=2, space="PSUM") as ps_st, \
             tc.tile_pool(name="ps_av", bufs=2, space="PSUM") as ps_av, \
             tc.tile_pool(name="epool", bufs=6) as epool, \
             tc.tile_pool(name="npool", bufs=2) as npool, \
             tc.tile_pool(name="osb", bufs=2) as osb:
            # remaining QKV chunks are emitted DURING iblock-0 attention
            # (one block per 4 attention blocks, borrowing an st3 slot) so
            # the exp stream starts ~80us earlier with no burst stalls
            qk_during_pair = {0: (1, 5), 1: (2, 6), 2: (3, 7)}
            for ib in range(NI):
                blk = slice(ib * IB, (ib + 1) * IB)
                for p in range(4):
                    side_work = []
                    if ib == 0 and p in qk_during_pair:
                        side_work = [(m, ibq) for m in qk_during_pair[p]
                                     for ibq in range(NI)]
                    qa = qk_sb[p]      # rows 0:64 head 2p, 64:128 head 2p+1
                    ka = qk_sb[4 + p]
                    avA = ps_av.tile([DH + 1, IB], F32, tag="avA", name="avA",
                                     bufs=1)
                    avB = ps_av.tile([DH + 1, IB], F32, tag="avB", name="avB",
                                     bufs=1)

                    def av_mm(eblk, b):
                        j, h = b // 2, b % 2
                        av = avB if h else avA
                        nc.tensor.matmul(av, lhsT=v_sb[j][:, 2 * p + h, :],
                                         rhs=eblk, start=(j == 0),
                                         stop=(j == NJ - 1))

                    # scores^T blocks b = 2j+head packed 3-per-PSUM-tile
                    # ([128, 1536] = 3 banks); exp has no per-key bias (mask
                    # lives in v), so blocks of different j share one exp.
                    NB = 2 * NJ
                    BPT = 3
                    pend = []
                    st3 = e3 = None
                    for b in range(NB):
                        j, h = b // 2, b % 2
                        s = b % BPT
                        if s == 0:
                            st3 = ps_st.tile([P, BPT * IB], F32, tag="st3",
                                             name="st3", bufs=2)
                        jcol = slice(j * P, (j + 1) * P)
                        hsl = slice(h * DH, h * DH + DH) if h else slice(0, DH)
                        nc.tensor.matmul(st3[:, s * IB:(s + 1) * IB],
                                         lhsT=ka[hsl, jcol],
                                         rhs=qa[hsl, blk],
                                         start=True, stop=True)
                        if b % 4 == 3 and side_work:
                            mq, ibq = side_work.pop(0)
                            qk_emitter["f"](mq, ibq, pool=ps_st)
                        if s == BPT - 1 or b == NB - 1:
                            w = (s + 1) * IB
                            e3 = epool.tile([P, BPT * IB], mmdt, tag="e3",
                                            name="e3")
                            nc.scalar.activation(e3[:, 0:w], st3[:, 0:w],
                                                 AFT.Exp,
                                                 scale=1.0 / math.sqrt(DH))
                            for bb in range(b - s, b + 1):
                                pend.append((e3[:, (bb - (b - s)) * IB:
                                                (bb - (b - s) + 1) * IB], bb))
                            while len(pend) > BPT:
                                av_mm(*pend.pop(0))
                    for it in pend:
                        av_mm(*it)
                    # normalize by the softmax denominator (row 64 of av):
                    # reciprocal -> PE K=1 ones-matmul broadcast -> multiply
                    rec2 = npool.tile([DH + 1, 2 * IB], F32, tag="rec2",
                                      name="rec2")
                    nc.vector.reciprocal(rec2[DH:DH + 1, 0:IB],
                                         avA[DH:DH + 1, :])
                    nc.vector.reciprocal(rec2[DH:DH + 1, IB:2 * IB],
                                         avB[DH:DH + 1, :])
                    bc = ps_st.tile([P, 3 * IB], F32, tag="st3", name="bc",
                                    bufs=2)
                    nc.tensor.matmul(bc[0:DH, 0:IB],
                                     lhsT=ones_sb[DH:DH + 1, :],
                                     rhs=rec2[DH:DH + 1, 0:IB],
                                     start=True, stop=True)
                    nc.tensor.matmul(bc[0:DH, IB:2 * IB],
                                     lhsT=ones_sb[DH:DH + 1, :],
                                     rhs=rec2[DH:DH + 1, IB:2 * IB],
                                     start=True, stop=True)
                    bc_sb = npool.tile([DH, 2 * IB], F32, tag="bc_sb",
                                       name="bc_sb")
                    nc.vector.tensor_copy(bc_sb, bc[0:DH, 0:2 * IB])
                    nc.vector.tensor_mul(attnoutT[p][0:DH, blk],
                                         avA[0:DH, :], bc_sb[:, 0:IB])
                    tb = npool.tile([DH, IB], mmdt, tag="tb", name="tb")
                    nc.vector.tensor_mul(tb, avB[0:DH, :], bc_sb[:, IB:2 * IB])
                    # move head B's rows to partitions 64:128 (SBUF->SBUF DMA)
                    nc.sync.dma_start(out=attnoutT[p][DH:P, blk], in_=tb)

                # out projection for this query block (host all-reduces pairs)
                for t in range(ib * IB // P, (ib + 1) * IB // P):
                    o = osb.tile([P, DIM], F32, tag="o", name="o")
                    for db in range(DIM // IB):
                        # alternate between the two 1-bank av slots so
                        # consecutive psum groups double-buffer
                        ptag = "avA" if (2 * t + db) % 2 == 0 else "avB"
                        pp = ps_av.tile([P, IB], F32, tag=ptag, name="pp",
                                        bufs=1)
                        for c in range(CH // P):
                            nc.tensor.matmul(pp[:, 0:IB],
                                             lhsT=attnoutT[c][:, t * P:(t + 1) * P],
                                             rhs=wo_sb[c][:, db * IB:(db + 1) * IB],
                                             start=(c == 0),
                                             stop=(c == CH // P - 1))
                        nc.vector.tensor_copy(o[:, db * IB:(db + 1) * IB],
                                              pp[:, 0:IB])
                    nc.sync.dma_start(out=out_d[t * P:(t + 1) * P, :], in_=o)

    # Drop same-engine waits on ACT instructions: ACT is strict-FIFO and
    # in-order, and no ACT op here reads another ACT op's output, so these
    # WAW slot-reuse waits (vs ops >=bufs back) are trivially satisfied.
    # Removing them keeps each exp at a single (PE) wait, avoiding the
    # EventSemaphore split that would otherwise cost ~100ns/exp on the
    # ACT critical path.
    for _bb in nc.m.functions[0].blocks:
        for _inst in _bb.instructions:
            if not str(getattr(_inst, 'engine', '')).endswith('Activation'):
                continue
            _si = _inst.sync_info
            if _si is None or len(_si.on_wait) < 2:
                continue
            _kept = [w for w in _si.on_wait
                     if not w.ant_name.startswith('Activation')]
            if _kept and len(_kept) < len(_si.on_wait):
                _si.on_wait = _kept

    nc.compile()
    return nc


_PROGRAM = None


def _get_program():
    global _PROGRAM
    if _PROGRAM is None:
        _PROGRAM = _build_program()
    return _PROGRAM


def _wrap_pi(a):
    return ((a + np.pi) % (2.0 * np.pi)) - np.pi


_LAST_RES = None


def _prepare_in_maps(inputs):
    x = np.asarray(inputs["x"], dtype=np.float32)
    mask = np.asarray(inputs["mask"])
    freqs = np.asarray(inputs["freqs"], dtype=np.float32)
    w_in = np.asarray(inputs["w_in"], dtype=np.float32)
    b_in = np.asarray(inputs["b_in"], dtype=np.float32)
    w_out = np.asarray(inputs["w_out"], dtype=np.float32)

    bf = ml_dtypes.bfloat16

    # rotate_half as a matrix: rh = R @ t, rh[2i] = -t[2i+1], rh[2i+1] = t[2i]
    R = np.zeros((DH, DH), np.float32)
    idx = np.arange(DH // 2)
    R[2 * idx, 2 * idx + 1] = -1.0
    R[2 * idx + 1, 2 * idx] = 1.0
    rt_host = np.ascontiguousarray(R.T).astype(bf)

    fT = freqs.T.astype(np.float32)                     # [64, N]
    zT = np.zeros_like(fT)
    freq_host = {}
    for hg in range(2):
        f = fT if hg == 0 else zT
        freq_host[hg] = (np.ascontiguousarray(_wrap_pi(f)),
                         np.ascontiguousarray(_wrap_pi(f + np.pi / 2)))

    # per-batch pieces (shared by the two head-group cores of each batch)
    xT_host, mb_host = {}, {}
    for b in range(B):
        xT_host[b] = np.ascontiguousarray(x[b].T).astype(bf)
        m01 = mask[b].astype(np.float32)
        mb_host[b] = np.ascontiguousarray(m01.reshape(NJ, P).T)

    # per-head-group pieces (shared by the four batch cores of each group)
    hg_host = {}
    for hg in range(2):
        sl = slice(CH * hg, CH * hg + CH)
        wq = w_in[0 * INNER:1 * INNER][sl]
        wk = w_in[1 * INNER:2 * INNER][sl]
        wv = w_in[2 * INNER:3 * INNER][sl]
        bq = b_in[0 * INNER:1 * INNER][sl]
        bk = b_in[1 * INNER:2 * INNER][sl]
        bv = b_in[2 * INNER:3 * INNER][sl]
        hg_host[hg] = {
            "wqkT": np.ascontiguousarray(np.concatenate([wq, wk], 0).T).astype(bf),
            "wvT": np.ascontiguousarray(wv.T).astype(bf),
            "woT": np.ascontiguousarray(w_out[:, sl].T).astype(bf),
            "bqk": np.ascontiguousarray(
                np.concatenate([bq, bk], 0).reshape(KD, P).T),
            "bv": np.ascontiguousarray(bv.reshape(1, CH)),
        }

    in_maps = []
    for c in range(NCORES):
        hg, b = c // B, c % B
        in_maps.append({
            "xT": xT_host[b],
            "fsin": freq_host[hg][0],
            "fcos": freq_host[hg][1],
            "rt": rt_host,
            "mb": mb_host[b],
            **hg_host[hg],
        })
    return in_maps


def kernel(x, mask, freqs, w_in, b_in, w_out, b_out, _trace=False):
    global _LAST_RES
    mask = np.asarray(mask)
    b_out = np.asarray(b_out, dtype=np.float32)
    nc = _get_program()
    in_maps = _prepare_in_maps(dict(x=x, mask=mask, freqs=freqs, w_in=w_in,
                                    b_in=b_in, w_out=w_out, b_out=b_out))

    res = run_bass_kernel_spmd(nc, in_maps, list(range(NCORES)), trace=_trace)
    _LAST_RES = res

    out = np.zeros((B, N, DIM), np.float32)
    for c in range(NCORES):
        out[c % B] += res.results[c]["out"]
    out += b_out[None, None, :]
    out *= mask[..., None].astype(np.float32)
    return out

